# revision 40
# baseline (speedup 1.0000x reference)
"""Trainium2 Bass kernel for nn_Aggregate: y = x @ W^T followed by an EMA scan
(h_s = 0.5*h_{s-1} + 0.5*y_s) over the segment axis, returning
(h_final [B, F_out], hiddens [B, S, F_out]).

Strategy
--------
Both stages are linear in x, and they commute:
    EMA_s(x @ W^T) = EMA_s(x) @ W^T
so the scan is applied FIRST, directly to x, as a banded matmul: with chunks
of 128 segments, h contributions decay as 0.5^lag and underflow to exactly 0
beyond lag 128+, so  u = EMA(x)  is computed per 128-chunk k as
    uT[:, k] = x_k^T @ L0 + x_{k-1}^T @ L1
with constant 128x128 decay matrices L0/L1 (exact powers of two). Feeding x as
the matmul *stationary* operand keeps x in its native [token, feature] layout
(no transposes anywhere on device) and produces u already transposed in
[feature, segment] layout - exactly what the big GEMM needs as its stationary
operand. The second stage is then a plain dense GEMM u^T-slices @ W^T-slices
accumulated over feature chunks in PSUM.

Sharding: data-parallel over batch, 8 batches per core x 8 cores. W^T and the
decay constants are replicated. Operands are cast to bf16 on the host (exact
for the power-of-two decay matrices); all accumulation is fp32 in PSUM; the
output is fp32.
"""

import numpy as np
import ml_dtypes

BF16 = ml_dtypes.bfloat16

B, S, F, O = 64, 512, 1024, 1024
N_CORES = 8
B_LOC = B // N_CORES          # 8 batches per core
P = 128                       # partitions / chunk size
SC = S // P                   # 4 segment chunks per batch
FC = F // P                   # 8 feature chunks
OH = O // 512                 # 2 output halves (PSUM bank = 512 fp32)
ALPHA = 0.5

_CACHE = {}


def _decay_constants():
    """L0T[j, s] = 0.5^(s-j+1) for s >= j (same-chunk lower triangle, transposed)
    L1T[j, s] = 0.5^(s+128-j+1)          (previous-chunk band, transposed)
    Built in fp64; every entry is a power of two so the bf16 cast is exact
    (entries below bf16's subnormal floor round to 0, which is ~1e-40 relative
    and invisible in fp32 outputs)."""
    j = np.arange(P)[:, None]
    s = np.arange(P)[None, :]
    l0t = np.where(s >= j, 0.5 ** (s - j + 1.0), 0.0)
    l1t = 0.5 ** (s + P - j + 1.0)
    return l0t.astype(BF16), l1t.astype(BF16)


def _build():
    """Build + compile the per-core Bass module (cached per process)."""
    if "nc" in _CACHE:
        return _CACHE["nc"]

    import concourse.bacc as bacc
    import concourse.tile as tile
    import concourse.mybir as mybir

    f32 = mybir.dt.float32
    bf16 = mybir.dt.bfloat16

    nc = bacc.Bacc("TRN2", debug=False, num_devices=N_CORES,
                   enable_partition_id=False)

    x_d = nc.dram_tensor("x", [B_LOC, S, F], bf16, kind="ExternalInput")
    wt_d = nc.dram_tensor("wt", [F, O], bf16, kind="ExternalInput")
    l01_d = nc.dram_tensor("l01t", [P, 2, P], bf16, kind="ExternalInput")
    h_d = nc.dram_tensor("h", [B_LOC, S, O], f32, kind="ExternalOutput")
    warm_d = nc.dram_tensor("warm", [P, 4], f32, kind="ExternalOutput")

    N_WARM = 50  # dummy matmuls bridging the initial DMA wait (HAM warm-up);
    # sized past the typical x_b0 landing so DMA jitter can't open a PE idle
    # gap that resets the clock-gate's sustained-busy window

    with tile.TileContext(nc) as tc:
        with (
            tc.tile_pool(name="const", bufs=1) as cpool,
            tc.tile_pool(name="xp", bufs=3) as xpool,
            tc.tile_pool(name="up", bufs=3) as upool,
            tc.tile_pool(name="op", bufs=6) as opool,
            tc.tile_pool(name="psA", bufs=4, space="PSUM") as psA,
            tc.tile_pool(name="psB", bufs=4, space="PSUM") as psB,
        ):
            # HAM pre-warm first: its weights come from a gpsimd memset (no
            # DMA wait), so PE activity starts right after the framework
            # preamble and the clock gate opens before real work arrives.
            # Emitted before any gpsimd-queue DMA so the memset isn't queued
            # behind multi-us DMA dispatches.
            warm_w = cpool.tile([P, P], bf16)
            nc.gpsimd.memset(warm_w[:], 0.0)
            # touch ScalarE's copy path now: its first ACTIVATE triggers a
            # ~1.5us activation-table load that must not land mid-scan
            act_warm = cpool.tile([P, 1], bf16)
            nc.scalar.copy(act_warm[:], warm_w[:, 0:1])
            # the warm bank borrows a GEMM psum slot (tag "ph"); once the
            # warm result is flushed at the end of b0, all 4 slots serve the
            # GEMM for the remaining batches
            ps_w = psB.tile([P, 512], f32, tag="ph", name="ps_warm")
            for w in range(N_WARM):
                nc.tensor.matmul(
                    ps_w[:, 0:P], warm_w[:], warm_w[:],
                    start=(w == 0), stop=(w == N_WARM - 1),
                )

            # constants: tiny, land in ~1us, unblock the first scan mms
            l01_sb = cpool.tile([P, 2, P], bf16)
            nc.sync.dma_start(l01_sb[:], l01_d.ap())
            l0_sb = l01_sb[:, 0, :]
            l1_sb = l01_sb[:, 1, :]

            # x for batch 0 next (the critical path), split by segment chunk
            # across two dispatch queues to halve dispatch serialization
            x_first = xpool.tile([P, SC, F], bf16)
            x_first_dmas = []
            for c in range(SC):
                eng = nc.sync if c % 2 == 0 else nc.gpsimd
                x_first_dmas.append(
                    eng.dma_start(
                        x_first[:, c, :],
                        x_d.ap()[0].rearrange("(c p) f -> p c f", p=P)[:, c, :],
                    )
                )

            # W^T gated behind x_b0: the HW DMA engines round-robin across
            # everything queued, so without the explicit dep wt's 2 MB would
            # steal bandwidth from the critical first x tile. First needed
            # when b0's GEMM starts (~15us), lands fc-chunk by fc-chunk.
            wt_sb = cpool.tile([P, FC, O], bf16)
            wt_r = wt_d.ap().rearrange("(fc p) o -> p fc o", p=P)
            wt_dmas = []
            for q in range(FC):
                wt_dmas.append(
                    nc.sync.dma_start(wt_sb[:, q, :], wt_r[:, q, :])
                )
                tile.add_dep_helper(
                    wt_dmas[-1].ins, x_first_dmas[-1].ins, sync=True,
                    reason="priority: x_b0 owns HBM bandwidth first",
                )

            # x tiles are prefetched one batch ahead (emitted at the top of
            # the previous iteration) so their dispatch never queues behind
            # output-DMA dispatches
            x_tiles = {0: x_first}

            def prefetch_x(b):
                if b < B_LOC and b not in x_tiles:
                    t = xpool.tile([P, SC, F], bf16, tag="x_first")
                    nc.sync.dma_start(
                        t[:], x_d.ap()[b].rearrange("(c p) f -> p c f", p=P)
                    )
                    x_tiles[b] = t

            for b in range(B_LOC):
                prefetch_x(b + 1)
                x_sb = x_tiles.pop(b)

                # ---- stage 1: uT[f, s] = EMA(x_b) transposed, per f-chunk ----
                def scan_mms(ps, fc, k):
                    """Emit the decay matmuls of segment-chunk k into bank ps."""
                    fs = slice(fc * P, (fc + 1) * P)
                    cs = slice(k * P, (k + 1) * P)
                    if k == 0:
                        nc.tensor.matmul(
                            ps[:, cs], x_sb[:, 0, fs], l0_sb[:],
                            start=True, stop=False,
                        )
                    else:
                        nc.tensor.matmul(
                            ps[:, cs], x_sb[:, k, fs], l0_sb[:],
                            start=False, stop=False,
                        )
                        nc.tensor.matmul(
                            ps[:, cs], x_sb[:, k - 1, fs], l1_sb[:],
                            start=False, stop=(k == SC - 1),
                        )

                def cast_ut(ps, fc, use_act):
                    # alternate psum drains between DVE and ScalarE when they
                    # are the pacing step (b0 has no GEMM to hide behind)
                    if use_act:
                        nc.scalar.copy(ut_sb[:, fc, :], ps[:])
                    else:
                        nc.vector.tensor_copy(ut_sb[:, fc, :], ps[:])

                ut_sb = upool.tile([P, FC, S], bf16)
                for fc in range(FC):
                    ps = psA.tile([P, S], f32)
                    for k in range(SC):
                        scan_mms(ps, fc, k)
                    # b0's scan has no GEMM to hide behind, so alternate its
                    # psum drains with the otherwise-idle ScalarE
                    cast_ut(ps, fc, use_act=(b == 0 and fc % 2 == 1))

                # ---- stage 2: h_b[s, o] = sum_f u[f, s] * WT[f, o] ----
                for sc in range(SC):
                    for oh in range(OH):
                        ph = psB.tile([P, 512], f32)
                        for fc in range(FC):
                            nc.tensor.matmul(
                                ph[:],
                                ut_sb[:, fc, sc * P:(sc + 1) * P],
                                wt_sb[:, fc, oh * 512:(oh + 1) * 512],
                                start=(fc == 0),
                                stop=(fc == FC - 1),
                            )
                        o_sb = opool.tile([P, 512], f32)
                        h_slice = h_d.ap()[b].rearrange("(c p) o -> p c o", p=P)[
                            :, sc, oh * 512:(oh + 1) * 512
                        ]
                        # split psum->sbuf drains across DVE and ScalarE so
                        # DVE keeps headroom for the scan casts; output DMAs
                        # dispatch from ScalarE's queue so their copy-waits
                        # never block x prefetch on sync. The very last group
                        # takes the idle DVE+sync path so its serial chain
                        # doesn't queue behind ScalarE's previous dispatch.
                        last = (b == B_LOC - 1 and sc == SC - 1 and oh == OH - 1)
                        if oh == 1 and not last:
                            nc.scalar.copy(o_sb[:], ph[:])
                        else:
                            nc.vector.tensor_copy(o_sb[:], ph[:])
                        if last:
                            nc.sync.dma_start(h_slice, o_sb[:])
                        else:
                            nc.scalar.dma_start(h_slice, o_sb[:])

                if b == 0:
                    # flush the warm-up result (exists only to defeat DCE)
                    # here: the copy is long since ready, so the dispatch
                    # doesn't stall the sync queue, and it stays off the tail
                    warm_sb = cpool.tile([P, 4], f32)
                    nc.vector.tensor_copy(warm_sb[:], ps_w[:, 0:4])
                    nc.sync.dma_start(warm_d.ap(), warm_sb[:])

    nc.compile()
    _CACHE["nc"] = nc
    return nc


def _prepare_inputs(x, W):
    """Host-side prep: cast to bf16, transpose W, shard x over cores."""
    xb = np.ascontiguousarray(x).astype(BF16)
    wt = np.ascontiguousarray(W.T).astype(BF16)
    l0t, l1t = _decay_constants()
    l01t = np.ascontiguousarray(np.stack([l0t, l1t], axis=1))  # [P, 2, P]
    in_maps = []
    for c in range(N_CORES):
        in_maps.append(
            {
                "x": np.ascontiguousarray(xb[c * B_LOC:(c + 1) * B_LOC]),
                "wt": wt,
                "l01t": l01t,
            }
        )
    return in_maps


def _run(x, W, trace=False):
    from concourse.bass_utils import run_bass_kernel_spmd

    nc = _build()
    in_maps = _prepare_inputs(x, W)
    res = None
    for attempt in range(3):
        res = run_bass_kernel_spmd(
            nc, in_maps, core_ids=list(range(N_CORES)), trace=trace
        )
        hiddens = np.concatenate(
            [res.results[c]["h"] for c in range(N_CORES)], axis=0
        )
        # transient transport/device glitches have been observed to surface
        # as NaNs in an otherwise NaN-free computation; retry those
        if not np.isnan(hiddens).any():
            break
    hk = np.ascontiguousarray(hiddens[:, -1, :])
    return (hk, hiddens), res


def kernel(x, W):
    out, _ = _run(x, W, trace=False)
    return out


# revision 41
# speedup vs baseline: 1.0173x; 1.0173x over previous
"""Trainium2 Bass kernel for nn_Aggregate: y = x @ W^T followed by an EMA scan
(h_s = 0.5*h_{s-1} + 0.5*y_s) over the segment axis, returning
(h_final [B, F_out], hiddens [B, S, F_out]).

Strategy
--------
Both stages are linear in x, and they commute:
    EMA_s(x @ W^T) = EMA_s(x) @ W^T
so the scan is applied FIRST, directly to x, as a banded matmul: with chunks
of 128 segments, h contributions decay as 0.5^lag and underflow to exactly 0
beyond lag 128+, so  u = EMA(x)  is computed per 128-chunk k as
    uT[:, k] = x_k^T @ L0 + x_{k-1}^T @ L1
with constant 128x128 decay matrices L0/L1 (exact powers of two). Feeding x as
the matmul *stationary* operand keeps x in its native [token, feature] layout
(no transposes anywhere on device) and produces u already transposed in
[feature, segment] layout - exactly what the big GEMM needs as its stationary
operand. The second stage is then a plain dense GEMM u^T-slices @ W^T-slices
accumulated over feature chunks in PSUM.

Sharding: data-parallel over batch, 8 batches per core x 8 cores. W^T and the
decay constants are replicated. Operands are cast to bf16 on the host (exact
for the power-of-two decay matrices); all accumulation is fp32 in PSUM; the
output is fp32.
"""

import numpy as np
import ml_dtypes

BF16 = ml_dtypes.bfloat16

B, S, F, O = 64, 512, 1024, 1024
N_CORES = 8
B_LOC = B // N_CORES          # 8 batches per core
P = 128                       # partitions / chunk size
SC = S // P                   # 4 segment chunks per batch
FC = F // P                   # 8 feature chunks
OH = O // 512                 # 2 output halves (PSUM bank = 512 fp32)
ALPHA = 0.5

_CACHE = {}


def _decay_constants():
    """L0T[j, s] = 0.5^(s-j+1) for s >= j (same-chunk lower triangle, transposed)
    L1T[j, s] = 0.5^(s+128-j+1)          (previous-chunk band, transposed)
    Built in fp64; every entry is a power of two so the bf16 cast is exact
    (entries below bf16's subnormal floor round to 0, which is ~1e-40 relative
    and invisible in fp32 outputs)."""
    j = np.arange(P)[:, None]
    s = np.arange(P)[None, :]
    l0t = np.where(s >= j, 0.5 ** (s - j + 1.0), 0.0)
    l1t = 0.5 ** (s + P - j + 1.0)
    return l0t.astype(BF16), l1t.astype(BF16)


def _build():
    """Build + compile the per-core Bass module (cached per process)."""
    if "nc" in _CACHE:
        return _CACHE["nc"]

    import concourse.bacc as bacc
    import concourse.tile as tile
    import concourse.mybir as mybir

    f32 = mybir.dt.float32
    bf16 = mybir.dt.bfloat16

    nc = bacc.Bacc("TRN2", debug=False, num_devices=N_CORES,
                   enable_partition_id=False)

    x_d = nc.dram_tensor("x", [B_LOC, S, F], bf16, kind="ExternalInput")
    wt_d = nc.dram_tensor("wt", [F, O], bf16, kind="ExternalInput")
    l01_d = nc.dram_tensor("l01t", [P, 2, P], bf16, kind="ExternalInput")
    h_d = nc.dram_tensor("h", [B_LOC, S, O], f32, kind="ExternalOutput")
    warm_d = nc.dram_tensor("warm", [P, 4], f32, kind="ExternalOutput")

    N_WARM = 40  # dummy matmuls bridging the initial DMA wait (HAM warm-up)

    with tile.TileContext(nc) as tc:
        with (
            tc.tile_pool(name="const", bufs=1) as cpool,
            tc.tile_pool(name="xp", bufs=3) as xpool,
            tc.tile_pool(name="up", bufs=3) as upool,
            tc.tile_pool(name="op", bufs=6) as opool,
            tc.tile_pool(name="psA", bufs=4, space="PSUM") as psA,
            tc.tile_pool(name="psB", bufs=4, space="PSUM") as psB,
        ):
            # HAM pre-warm first: its weights come from a gpsimd memset (no
            # DMA wait), so PE activity starts right after the framework
            # preamble and the clock gate opens before real work arrives.
            # Emitted before any gpsimd-queue DMA so the memset isn't queued
            # behind multi-us DMA dispatches.
            warm_w = cpool.tile([P, P], bf16)
            nc.gpsimd.memset(warm_w[:], 0.0)
            # touch ScalarE's copy path now: its first ACTIVATE triggers a
            # ~1.5us activation-table load that must not land mid-scan
            act_warm = cpool.tile([P, 1], bf16)
            nc.scalar.copy(act_warm[:], warm_w[:, 0:1])
            # the warm bank borrows a GEMM psum slot (tag "ph"); once the
            # warm result is flushed at the end of b0, all 4 slots serve the
            # GEMM for the remaining batches
            ps_w = psB.tile([P, 512], f32, tag="ph", name="ps_warm")
            for w in range(N_WARM):
                nc.tensor.matmul(
                    ps_w[:, 0:P], warm_w[:], warm_w[:],
                    start=(w == 0), stop=(w == N_WARM - 1),
                )

            # constants: tiny, land in ~1us, unblock the first scan mms
            l01_sb = cpool.tile([P, 2, P], bf16)
            nc.sync.dma_start(l01_sb[:], l01_d.ap())
            l0_sb = l01_sb[:, 0, :]
            l1_sb = l01_sb[:, 1, :]

            # x for batch 0 next (the critical path), split by segment chunk
            # across two dispatch queues to halve dispatch serialization
            x_first = xpool.tile([P, SC, F], bf16)
            x_first_dmas = []
            for c in range(SC):
                eng = nc.sync if c % 2 == 0 else nc.gpsimd
                x_first_dmas.append(
                    eng.dma_start(
                        x_first[:, c, :],
                        x_d.ap()[0].rearrange("(c p) f -> p c f", p=P)[:, c, :],
                    )
                )

            # W^T gated behind x_b0: the HW DMA engines round-robin across
            # everything queued, so without the explicit dep wt's 2 MB would
            # steal bandwidth from the critical first x tile. First needed
            # when b0's GEMM starts (~15us), lands fc-chunk by fc-chunk.
            wt_sb = cpool.tile([P, FC, O], bf16)
            wt_r = wt_d.ap().rearrange("(fc p) o -> p fc o", p=P)
            wt_dmas = []
            for q in range(FC):
                wt_dmas.append(
                    nc.sync.dma_start(wt_sb[:, q, :], wt_r[:, q, :])
                )
                tile.add_dep_helper(
                    wt_dmas[-1].ins, x_first_dmas[-1].ins, sync=True,
                    reason="priority: x_b0 owns HBM bandwidth first",
                )

            # x tiles are prefetched one batch ahead (emitted at the top of
            # the previous iteration) so their dispatch never queues behind
            # output-DMA dispatches
            x_tiles = {0: x_first}

            def prefetch_x(b):
                if b < B_LOC and b not in x_tiles:
                    t = xpool.tile([P, SC, F], bf16, tag="x_first")
                    nc.sync.dma_start(
                        t[:], x_d.ap()[b].rearrange("(c p) f -> p c f", p=P)
                    )
                    x_tiles[b] = t

            for b in range(B_LOC):
                prefetch_x(b + 1)
                x_sb = x_tiles.pop(b)

                # ---- stage 1: uT[f, s] = EMA(x_b) transposed, per f-chunk ----
                def scan_mms(ps, fc, k):
                    """Emit the decay matmuls of segment-chunk k into bank ps."""
                    fs = slice(fc * P, (fc + 1) * P)
                    cs = slice(k * P, (k + 1) * P)
                    if k == 0:
                        nc.tensor.matmul(
                            ps[:, cs], x_sb[:, 0, fs], l0_sb[:],
                            start=True, stop=False,
                        )
                    else:
                        nc.tensor.matmul(
                            ps[:, cs], x_sb[:, k, fs], l0_sb[:],
                            start=False, stop=False,
                        )
                        nc.tensor.matmul(
                            ps[:, cs], x_sb[:, k - 1, fs], l1_sb[:],
                            start=False, stop=(k == SC - 1),
                        )

                def cast_ut(ps, fc, use_act):
                    # alternate psum drains between DVE and ScalarE when they
                    # are the pacing step (b0 has no GEMM to hide behind)
                    if use_act:
                        nc.scalar.copy(ut_sb[:, fc, :], ps[:])
                    else:
                        nc.vector.tensor_copy(ut_sb[:, fc, :], ps[:])

                ut_sb = upool.tile([P, FC, S], bf16)
                for fc in range(FC):
                    ps = psA.tile([P, S], f32)
                    for k in range(SC):
                        scan_mms(ps, fc, k)
                    # b0's scan has no GEMM to hide behind, so alternate its
                    # psum drains with the otherwise-idle ScalarE
                    cast_ut(ps, fc, use_act=(b == 0 and fc % 2 == 1))

                # ---- stage 2: h_b[s, o] = sum_f u[f, s] * WT[f, o] ----
                for sc in range(SC):
                    for oh in range(OH):
                        ph = psB.tile([P, 512], f32)
                        for fc in range(FC):
                            nc.tensor.matmul(
                                ph[:],
                                ut_sb[:, fc, sc * P:(sc + 1) * P],
                                wt_sb[:, fc, oh * 512:(oh + 1) * 512],
                                start=(fc == 0),
                                stop=(fc == FC - 1),
                            )
                        o_sb = opool.tile([P, 512], f32)
                        h_slice = h_d.ap()[b].rearrange("(c p) o -> p c o", p=P)[
                            :, sc, oh * 512:(oh + 1) * 512
                        ]
                        # split psum->sbuf drains across DVE and ScalarE so
                        # DVE keeps headroom for the scan casts; output DMAs
                        # dispatch from ScalarE's queue so their copy-waits
                        # never block x prefetch on sync. The very last group
                        # takes the idle DVE+sync path so its serial chain
                        # doesn't queue behind ScalarE's previous dispatch.
                        last = (b == B_LOC - 1 and sc == SC - 1 and oh == OH - 1)
                        if oh == 1 and not last:
                            nc.scalar.copy(o_sb[:], ph[:])
                        else:
                            nc.vector.tensor_copy(o_sb[:], ph[:])
                        if last:
                            nc.sync.dma_start(h_slice, o_sb[:])
                        else:
                            nc.scalar.dma_start(h_slice, o_sb[:])

                if b == 0:
                    # flush the warm-up result (exists only to defeat DCE)
                    # here: the copy is long since ready, so the dispatch
                    # doesn't stall the sync queue, and it stays off the tail
                    warm_sb = cpool.tile([P, 4], f32)
                    nc.vector.tensor_copy(warm_sb[:], ps_w[:, 0:4])
                    nc.sync.dma_start(warm_d.ap(), warm_sb[:])

    nc.compile()
    _CACHE["nc"] = nc
    return nc


def _prepare_inputs(x, W):
    """Host-side prep: cast to bf16, transpose W, shard x over cores."""
    xb = np.ascontiguousarray(x).astype(BF16)
    wt = np.ascontiguousarray(W.T).astype(BF16)
    l0t, l1t = _decay_constants()
    l01t = np.ascontiguousarray(np.stack([l0t, l1t], axis=1))  # [P, 2, P]
    in_maps = []
    for c in range(N_CORES):
        in_maps.append(
            {
                "x": np.ascontiguousarray(xb[c * B_LOC:(c + 1) * B_LOC]),
                "wt": wt,
                "l01t": l01t,
            }
        )
    return in_maps


def _run(x, W, trace=False):
    from concourse.bass_utils import run_bass_kernel_spmd

    nc = _build()
    in_maps = _prepare_inputs(x, W)
    res = None
    for attempt in range(3):
        res = run_bass_kernel_spmd(
            nc, in_maps, core_ids=list(range(N_CORES)), trace=trace
        )
        hiddens = np.concatenate(
            [res.results[c]["h"] for c in range(N_CORES)], axis=0
        )
        # transient transport/device glitches have been observed to surface
        # as NaNs in an otherwise NaN-free computation; retry those
        if not np.isnan(hiddens).any():
            break
    hk = np.ascontiguousarray(hiddens[:, -1, :])
    return (hk, hiddens), res


def kernel(x, W):
    out, _ = _run(x, W, trace=False)
    return out
